# revision 6
# baseline (speedup 1.0000x reference)
"""DescriptorLoss kernel for Trainium2 (8 NeuronCores, SPMD data-parallel).

Math:
    d[b,ij,kl] = sum_c desc0[b,c,ij] * desc1[b,c,kl]
    loss = mean(where(mask, 250*relu(1 - d), relu(d - 0.2)))

Per core (shard = (batch, i-slab) -> 1024 ij rows x 4096 kl cols), the PE
computes d' = 5*d via fp8 matmuls into PSUM fp32, in 16 pairs of
[128 x 2048] (2 chunks of [128 x 1024] each).  In d' units the hinges sit
at 1 and 5:
    5*loss_elem = relu(d'-1)        if m == 0
                  250*relu(5-d')    if m == 1

11 pairs go to the DVE (one fused custom op per pair, Src1 = t =
(m ? 8192 : 1) fp8 e5m2):
    body = relu(d' - t) + relu((t - d' - 8187) * 250),  accum = sum
  m=0: relu(d'-1); m=1: 250*relu(5-d').  One PSUM read per element.

5 pairs go to ACT: the PE injects the mask into PSUM
(psum += (-8192*I).T @ m) giving dM = d' - 8192*m; ACT runs two relu
passes with the weights folded into the free affine:
    acc1 = sum relu(dM - 1)              = sum_{m=0} relu(d'-1)
    acc2 = sum relu(-250*dM - 250*8187)  = 250 * sum_{m=1} relu(5-d')

DMA: mask groups stream on two parallel rings (sync HWDGE + gpsimd
SWDGE) in consumption order, sizes (2,2,4,8,8,8) chunks; descriptors
(aw/bm) lead on the sync ring.  The scalar engine issues no DMAs so it
is free for the hinge passes.
"""

import numpy as np
import ml_dtypes
from operator import add

import concourse.bacc as bacc
import concourse.mybir as mybir
import concourse.tile as tile
import concourse.dve_ops as dve_ops_mod
from concourse.dve_spec import Spec, Src0, Src1, C0, C1, relu, lower
from concourse.dve_uop import DveOpSpec
from concourse.bass_utils import run_bass_kernel_spmd

B, D, H, W = 2, 128, 64, 64
N_CORES = 8
IJ = H * W                # 4096
ROWS_PER_CORE = IJ // 4   # 1024
G = ROWS_PER_CORE // 128  # 8 row groups of 128
CH = 1024                 # chunk columns
KT = IJ // CH             # 4 chunks per row group
N_CHUNKS = G * KT         # 32
N_PAIRS = N_CHUNKS // 2   # 16
MOFF = 8192.0             # mask offset (exact in fp8 e5m2)
LAM = 250.0
MM_FD = 512               # matmul moving free dim (one PSUM bank)

# pairs handled by the Scalar (ACT) engine; rest on the fused DVE op
ACT_PAIRS = (3, 6, 9, 12, 15)
DVE_PAIRS = tuple(p for p in range(N_PAIRS) if p not in ACT_PAIRS)
ACT_CHUNKS = frozenset(c for p in ACT_PAIRS for c in (2 * p, 2 * p + 1))

# mask DMA groups, in chunk order; ring: 0 = gpsimd (SWDGE), 1 = sync
MGROUPS = ((2, 0), (2, 0), (4, 0), (8, 1), (8, 0), (8, 1))

_cached = {}

_OP_NAME = "HINGE_PAIR_MASKED_ANT"


def _hinge_ref(in0, in1, s0, s1, imm2):
    x = in0.astype(np.float32)
    t = in1.astype(np.float32)
    out = np.maximum(x - t, 0) + np.maximum((t - x - s0) * s1, 0)
    return out, out.reshape(out.shape[0], -1).sum(axis=-1, keepdims=True).astype(
        np.float32
    )


def _register_dve_op():
    """Register the fused two-hinge op in dve_ops.OPS (documented extension
    point; the uop table is emitted per-NEFF at compile time)."""
    for op in dve_ops_mod.OPS:
        if op.name == _OP_NAME:
            return op
    spec = Spec(
        body=relu(Src0 - Src1) + relu((Src1 - Src0 - C0) * C1),
        accum=add,
        reference=_hinge_ref,
    )
    opcode = dve_ops_mod._CUSTOM_DVE_ROW_BASE + len(dve_ops_mod.OPS)
    shas = {}
    for ver in ("v3", "v4"):
        shas[ver] = DveOpSpec(
            name=_OP_NAME, opcode=opcode, uops=lower(spec, ver=ver), rd1_en=True
        ).sha(ver)
    op = dve_ops_mod.DveOp(_OP_NAME, spec, subdim=False, uops_sha=shas)
    dve_ops_mod.OPS.append(op)
    dve_ops_mod._SUB_OPCODE_FOR_NAME[_OP_NAME] = opcode
    dve_ops_mod.CUSTOM_DVE_SPECS[_OP_NAME] = spec
    return op


_HINGE_OP = _register_dve_op()


def _build_program():
    nc = bacc.Bacc("TRN2")
    f32 = mybir.dt.float32
    bf16 = mybir.dt.bfloat16
    f8 = mybir.dt.float8e5
    f8e4 = mybir.dt.float8e4
    Act = mybir.ActivationFunctionType

    aw = nc.declare_dram_parameter("aw", [D, ROWS_PER_CORE], f8e4, isOutput=False)
    bm = nc.declare_dram_parameter("bm", [D, IJ], f8e4, isOutput=False)
    mvs = [
        nc.declare_dram_parameter(f"mv{i}", [128, n * CH], f8, isOutput=False)
        for i, (n, _) in enumerate(MGROUPS)
    ]
    idn = nc.declare_dram_parameter("idn", [D, D], f8, isOutput=False)
    accs_out = nc.declare_dram_parameter("accs", [128, 32], f32, isOutput=True)

    # group start (in chunks) for each mask group
    moff = []
    off = 0
    for n, _ in MGROUPS:
        moff.append(off)
        off += n

    with tile.TileContext(nc) as tc:
        with (
            tc.tile_pool(name="desc", bufs=1) as desc_pool,
            tc.tile_pool(name="mask", bufs=6) as mask_pool,
            tc.tile_pool(name="scr", bufs=4) as scr_pool,
            tc.tile_pool(name="accs", bufs=1) as acc_pool,
            tc.tile_pool(name="psd", bufs=2, space="PSUM") as psum_pool,
        ):
            mgrp = [
                mask_pool.tile([128, n * CH], f8, tag="m", name=f"mg{gq}")
                for gq, (n, _) in enumerate(MGROUPS)
            ]
            a_t = desc_pool.tile([D, ROWS_PER_CORE], f8e4, tag="a")
            b_t = desc_pool.tile([D, IJ], f8e4, tag="b")
            id_t = desc_pool.tile([D, D], f8, tag="idn")
            warm = desc_pool.tile([128, 8], bf16, tag="warm")
            warm2 = desc_pool.tile([128, 8], bf16, tag="warm2")
            bias_a = desc_pool.tile([128, 1], f32, tag="ba")
            bias_b = desc_pool.tile([128, 1], f32, tag="bb")
            acc_t = acc_pool.tile([128, 32], f32, tag="accs")

            # DMA ring 1 (sync/HWDGE): descriptors first, then late mask groups
            nc.sync.dma_start(a_t[:], aw[:])
            nc.sync.dma_start(b_t[:, 0:CH], bm[:, 0:CH])
            nc.sync.dma_start(b_t[:, CH:], bm[:, CH:])
            nc.sync.dma_start(id_t[:], idn[:])
            # DMA ring 2 (gpsimd/SWDGE): early mask groups
            for gq, (n, ring) in enumerate(MGROUPS):
                eng = nc.sync if ring else nc.gpsimd
                eng.dma_start(mgrp[gq][:], mvs[gq][:])

            # prime the ACT relu table (~2.7us one-time) under the input DMAs
            nc.vector.memset(warm[:], 0.0)
            nc.vector.memset(bias_a[:], -1.0)
            nc.vector.memset(bias_b[:], -(LAM * (MOFF - 5.0)))
            nc.scalar.activation(warm2[:], warm[:], Act.Relu, bias=bias_a[:], scale=1.0)

            # zero the padding columns of the accumulator output
            nc.vector.memset(acc_t[:, 21:], 0.0)

            n_dve = 0
            n_act = 0
            for p in range(N_PAIRS):
                on_act = p in ACT_PAIRS
                g = (2 * p) // KT
                rs = slice(g * 128, (g + 1) * 128)
                gq = max(i for i, o in enumerate(moff) if o <= 2 * p)
                mcol = (2 * p - moff[gq]) * CH
                mm_t = mgrp[gq][:, mcol:mcol + 2 * CH]

                psum_d = psum_pool.tile([128, 2 * CH], f32, tag="d")
                for s in range(2 * CH // MM_FD):
                    hs = slice(s * MM_FD, (s + 1) * MM_FD)
                    h0 = ((2 * p) % KT) * CH + s * MM_FD
                    nc.tensor.matmul(
                        psum_d[:, hs], a_t[:, rs], b_t[:, h0:h0 + MM_FD],
                        start=True, stop=not on_act,
                    )
                if on_act:
                    for s in range(2 * CH // MM_FD):
                        hs = slice(s * MM_FD, (s + 1) * MM_FD)
                        nc.tensor.matmul(
                            psum_d[:, hs], id_t[:], mm_t[:, hs],
                            start=False, stop=True,
                        )
                    scr1 = scr_pool.tile([128, 2 * CH], bf16, tag="scr")
                    scr2 = scr_pool.tile([128, 2 * CH], bf16, tag="scr")
                    c0 = 11 + 2 * n_act
                    nc.scalar.activation(
                        scr1[:], psum_d[:], Act.Relu,
                        bias=bias_a[:], scale=1.0,
                        accum_out=acc_t[:, c0:c0 + 1],
                    )
                    nc.scalar.activation(
                        scr2[:], psum_d[:], Act.Relu,
                        bias=bias_b[:], scale=-LAM,
                        accum_out=acc_t[:, c0 + 1:c0 + 2],
                    )
                    n_act += 1
                else:
                    scr = scr_pool.tile([128, 2 * CH], bf16, tag="scr")
                    nc.vector._custom_dve(
                        _HINGE_OP,
                        out=scr[:], in0=psum_d[:], in1=mm_t[:],
                        s0=MOFF - 5.0, s1=LAM,
                        accum_out=acc_t[:, n_dve:n_dve + 1],
                    )
                    n_dve += 1

            nc.sync.dma_start(accs_out[:], acc_t[:])

    nc.finalize()
    return nc


def _prep_inputs(descriptors_0, descriptors_1, similarity_mask):
    d0 = np.asarray(descriptors_0, dtype=np.float32)
    d1 = np.asarray(descriptors_1, dtype=np.float32)
    mkv = np.asarray(similarity_mask)
    idn = (-MOFF * np.eye(D, dtype=np.float32)).astype(ml_dtypes.float8_e5m2)
    in_maps = []
    for c in range(N_CORES):
        b = c >> 2
        isl = (c & 3) * 16
        aw = (d0[b].reshape(D, IJ)[:, isl * W:(isl + 16) * W] * np.float32(5.0)).astype(
            ml_dtypes.float8_e4m3
        )
        bmv = d1[b].reshape(D, IJ).astype(ml_dtypes.float8_e4m3)
        m = mkv[b, isl:isl + 16].reshape(ROWS_PER_CORE, IJ)
        # chunk tiles [32, 128, CH]: chunk cid = g*KT + h
        mq = m.reshape(G, 128, KT, CH).transpose(0, 2, 1, 3).reshape(N_CHUNKS, 128, CH)
        mvc = np.empty((N_CHUNKS, 128, CH), dtype=ml_dtypes.float8_e5m2)
        for cid in range(N_CHUNKS):
            if cid in ACT_CHUNKS:
                mvc[cid] = mq[cid].astype(ml_dtypes.float8_e5m2)
            else:
                mvc[cid] = np.where(
                    mq[cid], np.float32(MOFF), np.float32(1.0)
                ).astype(ml_dtypes.float8_e5m2)
        mvv = {}
        off = 0
        for i, (n, _) in enumerate(MGROUPS):
            grp = mvc[off:off + n]  # [n, 128, CH]
            mvv[f"mv{i}"] = np.ascontiguousarray(
                grp.transpose(1, 0, 2).reshape(128, n * CH)
            )
            off += n
        im = {
            "aw": np.ascontiguousarray(aw),
            "bm": np.ascontiguousarray(bmv),
            "idn": np.ascontiguousarray(idn),
        }
        im.update(mvv)
        in_maps.append(im)
    return in_maps


def _run(in_maps, **kwargs):
    if "nc" not in _cached:
        _cached["nc"] = _build_program()
    return run_bass_kernel_spmd(_cached["nc"], in_maps, list(range(N_CORES)), **kwargs)


def _combine(results):
    total = 0.0
    for r in results:
        accs = r["accs"].astype(np.float64)
        total += accs[:, :21].sum()
    return np.float32(total / 5.0 / float(B * IJ * IJ))


def kernel(descriptors_0, descriptors_1, similarity_mask):
    in_maps = _prep_inputs(descriptors_0, descriptors_1, similarity_mask)
    res = _run(in_maps)
    return _combine(res.results)
